# revision 7
# baseline (speedup 1.0000x reference)
"""Cross-attention Trainium2 Bass kernel (fp8 DoubleRow version).

Reference computation (per batch b):
    q = relu(scale_q * (Wq @ qf) + bias_q)          [C, Nq]
    k = relu(scale_k * (Wk @ kf) + bias_k)          [C, Nk]
    v = relu(scale_v * (Wv @ kf) + bias_v)          [C, Nk]
    sim  = q.T @ k / sqrt(C)                        [Nq, Nk]
    attn = softmax(sim, axis=-1)
    ctx  = v @ attn.T                               [C, Nq]

Sharding: 8 cores = 4 batches x 2 query halves (Nq 4096 -> 2048 per core).
Each core gets the full K/V for its batch (recomputed, cheap) and half the
query positions; output halves are concatenated on the host.

Device-side design (per core), all-fp8 (e4m3) matmul datapath:
  - Inputs, weights (BN scale folded on host), q/k/v activations and the
    exp(sim) tiles are all fp8e4m3; every big matmul runs in DoubleRow
    perf mode (256-deep contraction per instruction, 0.5 cycles/row).
    Measured end-to-end rel err of the full fp8 chain vs the fp32
    reference is ~5e-3 (threshold 2e-2).
  - sim is computed transposed (k on partitions, q on free dim) so the
    exp(sim) tiles feed the ctx matmul as the moving operand directly.
  - softmax uses a constant shift instead of a row max: exp(sim/sqrt(C) - 4)
    (sim/sqrt(C) is bounded by ~|q||k|/16 << 88, so no overflow), and the
    row sums come from a DoubleRow matmul with an all-ones fp8 stationary.
  - bias_v (which varies along the free dim of the vT tiles) is added via a
    rank-1 matmul (ones lhsT x bias rhs) accumulated into the same PSUM
    tile, so no separate DVE/ACT bias pass is needed.
  - ctx is accumulated unnormalized; normalization multiplies by 1/sums
    (reciprocal_approx_fast, ~18 bits) broadcast across partitions via a
    K=1 fp32 matmul with a ones column.
"""

import sys

for _p in ("/opt/trn_rl_repo", "/root/.axon_site/_ro/trn_rl_repo"):
    if _p not in sys.path:
        sys.path.insert(0, _p)

import ml_dtypes
import numpy as np

import concourse.bacc as bacc
import concourse.mybir as mybir
import concourse.tile as tile
from concourse.bass_utils import run_bass_kernel_spmd

F32 = mybir.dt.float32
F8 = mybir.dt.float8e4
AF = mybir.ActivationFunctionType
ALU = mybir.AluOpType
DR = mybir.MatmulPerfMode.DoubleRow
E4 = ml_dtypes.float8_e4m3

B, C, H, W = 4, 256, 64, 64
NK = H * W          # 4096 key positions per batch
NQ = NK // 2        # 2048 query positions per core
P = 128
CO = C // P         # 2 contraction subtiles (DoubleRow pair)
QC = 512            # query chunk (matmul moving free dim)
NQC = NQ // QC      # 4 query chunks per core
KT = NK // P        # 32 key tiles
NP = KT // 2        # 16 key-tile pairs
EXP_SHIFT = -4.0    # exp(sim/sqrt(C) + EXP_SHIFT); sim/sqrt(C) observed in [0.5, 7.7]
SCALE = 1.0 / np.sqrt(C)


def _build_program():
    nc = bacc.Bacc("TRN2", target_bir_lowering=False, debug=False)

    qf = nc.dram_tensor("qf", [C, NQ], F8, kind="ExternalInput").ap()
    kf = nc.dram_tensor("kf", [C, NK], F8, kind="ExternalInput").ap()
    wqT = nc.dram_tensor("wqT", [C, C], F8, kind="ExternalInput").ap()
    wkT = nc.dram_tensor("wkT", [C, C], F8, kind="ExternalInput").ap()
    wvT = nc.dram_tensor("wvT", [C, C], F8, kind="ExternalInput").ap()
    bq = nc.dram_tensor("bq", [P, CO], F32, kind="ExternalInput").ap()
    bk = nc.dram_tensor("bk", [P, CO], F32, kind="ExternalInput").ap()
    bv8 = nc.dram_tensor("bv8", [1, C], F8, kind="ExternalInput").ap()
    ones8 = nc.dram_tensor("ones8", [1, P], F8, kind="ExternalInput").ap()
    ones8p = nc.dram_tensor("ones8p", [P, 2 * 16], F8, kind="ExternalInput").ap()
    ones32 = nc.dram_tensor("ones32", [1, P], F32, kind="ExternalInput").ap()
    out = nc.dram_tensor("out", [C, NQ], F32, kind="ExternalOutput").ap()
    out_t = out.rearrange("(co ci) n -> ci co n", ci=P)

    with tile.TileContext(nc) as tc:
        with (
            nc.allow_low_precision(reason="fp8 matmul datapath (e4m3)"),
            tc.tile_pool(name="consts", bufs=1) as consts,
            tc.tile_pool(name="persist", bufs=1) as persist,
        ):
            # ---- constants (first projection only needs wqT + bq + qf) ----
            wqT_sb = consts.tile([P, CO, C], F8, name="wqT_sb")
            nc.gpsimd.dma_start(wqT_sb[:], wqT.rearrange("(co ci) o -> ci co o", ci=P))
            bq_sb = consts.tile([P, CO], F32, name="bq_sb")
            wkT_sb = consts.tile([P, CO, C], F8, name="wkT_sb")
            wvT_sb = consts.tile([P, CO, C], F8, name="wvT_sb")
            bk_sb = consts.tile([P, CO], F32, name="bk_sb")
            bv8_sb = consts.tile([1, C], F8, name="bv8_sb")
            ones8_sb = consts.tile([1, P], F8, name="ones8_sb")
            ones8p_sb = consts.tile([P, 2, 16], F8, name="ones8p_sb")
            ones32_sb = consts.tile([1, P], F32, name="ones32_sb")
            b0_sb = consts.tile([P, 1], F32, name="b0_sb")
            nc.vector.memset(b0_sb[:], EXP_SHIFT)
            # dummy activation: pulls the ~1.3us LoadActFuncSet into the
            # initial DMA-wait window instead of blocking the first relu
            warm_sb = consts.tile([P, 1], F32, name="warm_sb")
            nc.scalar.activation(warm_sb[:], b0_sb[:], AF.Relu)

            # ---- persistent activations (all fp8) ----
            q8_sb = persist.tile([P, CO, NQ], F8, name="q8_sb")
            k8_sb = persist.tile([P, CO, NK], F8, name="k8_sb")
            v8_sb = persist.tile([P, KT, C], F8, name="v8_sb")

            # ---- projections ----
            with (
                tc.tile_pool(name="staging", bufs=1) as staging,
                tc.tile_pool(name="proj_ps", bufs=1, space="PSUM") as proj_ps,
            ):
                qf_sb = staging.tile([P, CO, NQ], F8, name="qf_sb")
                qf_t = qf.rearrange("(co ci) n -> ci co n", ci=P)
                kf_sb = staging.tile([P, CO, NK], F8, name="kf_sb")
                kf_t = kf.rearrange("(co ci) n -> ci co n", ci=P)
                nc.gpsimd.dma_start(bq_sb[:], bq[:])
                nc.gpsimd.dma_start(wkT_sb[:], wkT.rearrange("(co ci) o -> ci co o", ci=P))
                nc.gpsimd.dma_start(bk_sb[:], bk[:])
                nc.gpsimd.dma_start(wvT_sb[:], wvT.rearrange("(co ci) o -> ci co o", ci=P))
                nc.gpsimd.dma_start(bv8_sb[:], bv8[:])
                nc.gpsimd.dma_start(ones8_sb[:], ones8[:])
                nc.gpsimd.dma_start(ones8p_sb[:], ones8p.rearrange("p (a b) -> p a b", a=2))
                nc.gpsimd.dma_start(ones32_sb[:], ones32[:])
                nc.sync.dma_start(qf_sb[:, :, :QC], qf_t[:, :, :QC])
                nc.sync.dma_start(qf_sb[:, :, QC:], qf_t[:, :, QC:])
                nc.sync.dma_start(kf_sb[:, :, :NK // 2], kf_t[:, :, :NK // 2])
                nc.sync.dma_start(kf_sb[:, :, NK // 2:], kf_t[:, :, NK // 2:])

                def proj_iter(j, w_sb, bias_sb, dst, src_sb):
                    # one [*, QC] chunk of a q/k projection; relu+bias for
                    # oo=0 runs on ACT, oo=1 on DVE so neither engine
                    # rate-limits PE
                    js = slice(j * QC, (j + 1) * QC)
                    for oo in range(CO):
                        ps = proj_ps.tile([P, QC], F32, tag="pj", bufs=2,
                                          name=f"ps_{j}_{oo}")
                        nc.tensor.matmul(
                            ps[:],
                            w_sb[:, :, oo * P:(oo + 1) * P],
                            src_sb[:, :, js],
                            start=True, stop=True, perf_mode=DR,
                        )
                        if oo == 0:
                            nc.scalar.activation(
                                dst[:, oo, js], ps[:], AF.Relu,
                                bias=bias_sb[:, oo:oo + 1],
                            )
                        else:
                            nc.vector.tensor_scalar(
                                dst[:, oo, js], ps[:],
                                bias_sb[:, oo:oo + 1], 0.0,
                                ALU.add, ALU.max,
                            )

                def vt_one(kt):
                    # vT[k, c] = relu(kf.T @ Wv'.T + bias_v): one DoubleRow
                    # matmul (256-deep contraction) + a rank-1 bias matmul
                    # accumulated into the same PSUM tile; relu alternates
                    # ACT/DVE.
                    psv = proj_ps.tile([P, C], F32, tag="pv", bufs=4,
                                       name=f"psv_{kt}")
                    nc.tensor.matmul(
                        psv[:],
                        kf_sb[:, :, kt * P:(kt + 1) * P],
                        wvT_sb[:, :, :],
                        start=True, stop=False, perf_mode=DR,
                    )
                    nc.tensor.matmul(
                        psv[:], ones8_sb[:], bv8_sb[:],
                        start=False, stop=True,
                    )
                    if kt % 2 == 0:
                        nc.scalar.activation(v8_sb[:, kt, :], psv[:], AF.Relu)
                    else:
                        nc.vector.tensor_scalar_max(v8_sb[:, kt, :], psv[:], 0.0)

                for j in range(NQ // QC):
                    proj_iter(j, wqT_sb, bq_sb, q8_sb, qf_sb)
                for j in range(NK // QC):
                    proj_iter(j, wkT_sb, bk_sb, k8_sb, kf_sb)
                    for kt in range(4 * j, 4 * j + 4):
                        vt_one(kt)

            # ---- attention ----
            with (
                tc.tile_pool(name="expp", bufs=1) as expp,
                tc.tile_pool(name="outp", bufs=1) as outp,
                tc.tile_pool(name="attn_ps", bufs=1, space="PSUM") as attn_ps,
            ):
                # Software pipeline: step s emits sim+exp for chunk s
                # interleaved (at k-pair granularity) with the ctx/sums
                # matmuls consuming chunk s-1's exp tiles. Adjacent exp pairs
                # are also merged (fp8 add) on the otherwise-idle Pool/DVE
                # engines so the row-sum matmuls only run per QUAD of key
                # tiles -- half the PE cost of summing per pair.
                NMQ = NP // 2          # 8 merged quads per chunk
                exp_pairs = {}         # qc -> list of pair tiles
                exp_quads = {}         # qc -> list of merged quad tiles

                def emit_sim_pair(qc, kp):
                    qs = slice(qc * QC, (qc + 1) * QC)
                    ps = attn_ps.tile([P, 2, QC], F32, tag="sim", bufs=2,
                                      name=f"pss_{qc}_{kp}")
                    for half in range(2):
                        kt = 2 * kp + half
                        nc.tensor.matmul(
                            ps[:, half, :],
                            k8_sb[:, :, kt * P:(kt + 1) * P],
                            q8_sb[:, :, qs],
                            start=True, stop=True, perf_mode=DR,
                        )
                    et = expp.tile([P, 2, QC], F8, tag="expT", bufs=20,
                                   name=f"expT_{qc}_{kp}")
                    nc.scalar.activation(et[:], ps[:], AF.Exp,
                                         bias=b0_sb[:], scale=float(SCALE))
                    pairs = exp_pairs.setdefault(qc, [])
                    pairs.append(et)
                    if kp % 2 == 1:
                        mg = expp.tile([P, 2, QC], F8, tag="mrg", bufs=12,
                                       name=f"mrg_{qc}_{kp // 2}")
                        eng = nc.gpsimd if (kp // 2) % 2 == 0 else nc.vector
                        eng.tensor_tensor(mg[:], pairs[kp - 1][:], pairs[kp][:],
                                          ALU.add)
                        exp_quads.setdefault(qc, []).append(mg)

                def emit_ctx_pair(qc, kp, ctx_ps, sums_ps,
                                  cts=(0, 1), with_sums=True):
                    e = exp_pairs[qc][kp]
                    for ct in cts:
                        nc.tensor.matmul(
                            ctx_ps[ct][:],
                            v8_sb[:, 2 * kp:2 * kp + 2, ct * P:(ct + 1) * P],
                            e[:],
                            start=(kp == 0), stop=(kp == NP - 1),
                            perf_mode=DR, skip_group_check=True,
                        )
                    if with_sums and kp % 2 == 1:
                        mq = kp // 2
                        nc.tensor.matmul(
                            sums_ps[:], ones8p_sb[:, :, :2],
                            exp_quads[qc][mq][:],
                            start=(mq == 0), stop=(mq == NMQ - 1),
                            perf_mode=DR, skip_group_check=True,
                        )

                def emit_norm_chain(qc, sums_ps):
                    # recip -> broadcast -> copy; runs as soon as the sums
                    # accumulation stops, overlapping remaining ctx matmuls
                    recip = outp.tile([1, QC], F32, tag="recip", bufs=2,
                                      name=f"recip_{qc}")
                    nc.vector.reciprocal_approx_fast(recip[:], sums_ps[0:1, :])
                    bc_ps = attn_ps.tile([P, QC], F32, tag="bc", bufs=1,
                                         name=f"psb_{qc}")
                    nc.tensor.matmul(bc_ps[:], ones32_sb[:], recip[:],
                                     start=True, stop=True)
                    bc_sb = outp.tile([P, QC], F32, tag="bc", bufs=2,
                                      name=f"bc_{qc}")
                    nc.vector.tensor_copy(out=bc_sb[:], in_=bc_ps[:])
                    return bc_sb

                def emit_out(qc, ct, ctx_ps, bc_sb):
                    qs = slice(qc * QC, (qc + 1) * QC)
                    ot = outp.tile([P, QC], F32, tag="out", bufs=3,
                                   name=f"out_{qc}_{ct}")
                    nc.vector.tensor_mul(ot[:], ctx_ps[ct][:], bc_sb[:])
                    nc.sync.dma_start(out_t[:, ct, qs], ot[:])

                ctx_live = None  # (qc, ctx_ps, sums_ps) being accumulated
                for s in range(NQC + 1):
                    if s > 0:
                        qcp = s - 1
                        ctx_ps = [
                            attn_ps.tile([P, QC], F32, tag="ctx", bufs=2,
                                         name=f"psc_{qcp}_{ct}")
                            for ct in range(CO)
                        ]
                        sums_ps = attn_ps.tile([2, QC], F32, tag="sums", bufs=1,
                                               name=f"psS_{qcp}")
                        ctx_live = (qcp, ctx_ps, sums_ps)
                    if s < NQC:
                        # steady state: sim pairs interleaved with prev
                        # chunk's ctx pairs + quad sums
                        for kp in range(NP):
                            emit_sim_pair(s, kp)
                            if ctx_live is not None:
                                emit_ctx_pair(ctx_live[0], kp, ctx_live[1],
                                              ctx_live[2])
                        if ctx_live is not None:
                            qcp, ctx_ps, sums_ps = ctx_live
                            bc_sb = emit_norm_chain(qcp, sums_ps)
                            for ct in range(CO):
                                emit_out(qcp, ct, ctx_ps, bc_sb)
                            exp_pairs.pop(qcp)
                            exp_quads.pop(qcp)
                            ctx_live = None
                    else:
                        # drain step (no sim work left): run all quad sums
                        # first so the norm chain overlaps the ctx matmuls,
                        # then finish ct=0 completely so its output DMA
                        # overlaps ct=1's matmuls.
                        qcp, ctx_ps, sums_ps = ctx_live
                        for mq in range(NMQ):
                            nc.tensor.matmul(
                                sums_ps[:], ones8p_sb[:, :, :2],
                                exp_quads[qcp][mq][:],
                                start=(mq == 0), stop=(mq == NMQ - 1),
                                perf_mode=DR, skip_group_check=True,
                            )
                        bc_sb = emit_norm_chain(qcp, sums_ps)
                        for ct in range(CO):
                            for kp in range(NP):
                                emit_ctx_pair(qcp, kp, ctx_ps, sums_ps,
                                              cts=(ct,), with_sums=False)
                            emit_out(qcp, ct, ctx_ps, bc_sb)
                        exp_pairs.pop(qcp)
                        exp_quads.pop(qcp)
                        ctx_live = None

    nc.compile()
    return nc


_PROGRAM = None


def _get_program():
    global _PROGRAM
    if _PROGRAM is None:
        _PROGRAM = _build_program()
    return _PROGRAM


def _prepare_in_maps(
    query_feats, key_feats, Wq, Wk, Wv,
    scale_q, bias_q, scale_k, bias_k, scale_v, bias_v,
):
    f32 = np.float32
    qf_all = np.asarray(query_feats, f32).reshape(B, C, NK)
    kf_all = np.asarray(key_feats, f32).reshape(B, C, NK)

    wqT = np.ascontiguousarray(
        (np.asarray(scale_q, f32)[:, None] * np.asarray(Wq, f32)).T).astype(E4)
    wkT = np.ascontiguousarray(
        (np.asarray(scale_k, f32)[:, None] * np.asarray(Wk, f32)).T).astype(E4)
    wvT = np.ascontiguousarray(
        (np.asarray(scale_v, f32)[:, None] * np.asarray(Wv, f32)).T).astype(E4)
    bq2 = np.ascontiguousarray(np.asarray(bias_q, f32).reshape(CO, P).T)
    bk2 = np.ascontiguousarray(np.asarray(bias_k, f32).reshape(CO, P).T)
    bv8 = np.asarray(bias_v, f32)[None, :].astype(E4)
    ones8 = np.ones((1, P), E4)
    ones8p = np.ones((P, 2 * 16), E4)
    ones32 = np.ones((1, P), f32)

    shared = dict(wqT=wqT, wkT=wkT, wvT=wvT, bq=bq2, bk=bk2,
                  bv8=bv8, ones8=ones8, ones8p=ones8p, ones32=ones32)
    in_maps = []
    for core in range(8):
        b, h = divmod(core, 2)
        in_maps.append(dict(
            qf=np.ascontiguousarray(
                qf_all[b][:, h * NQ:(h + 1) * NQ]).astype(E4),
            kf=np.ascontiguousarray(kf_all[b]).astype(E4),
            **shared,
        ))
    return in_maps


def run(inputs: dict, trace: bool = False):
    """Compile (cached) + run on 8 cores. Returns (output, BassKernelResults)."""
    nc = _get_program()
    in_maps = _prepare_in_maps(**inputs)
    res = run_bass_kernel_spmd(nc, in_maps, core_ids=list(range(8)), trace=trace)
    full = np.empty((B, C, NK), np.float32)
    for core in range(8):
        b, h = divmod(core, 2)
        full[b][:, h * NQ:(h + 1) * NQ] = res.results[core]["out"]
    return full.reshape(B, C, H, W), res


def kernel(**inputs) -> np.ndarray:
    return run(inputs)[0]


# revision 9
# speedup vs baseline: 1.2809x; 1.2809x over previous
"""Cross-attention Trainium2 Bass kernel (fp8 DoubleRow version).

Reference computation (per batch b):
    q = relu(scale_q * (Wq @ qf) + bias_q)          [C, Nq]
    k = relu(scale_k * (Wk @ kf) + bias_k)          [C, Nk]
    v = relu(scale_v * (Wv @ kf) + bias_v)          [C, Nk]
    sim  = q.T @ k / sqrt(C)                        [Nq, Nk]
    attn = softmax(sim, axis=-1)
    ctx  = v @ attn.T                               [C, Nq]

Sharding: 8 cores = 4 batches x 2 query halves (Nq 4096 -> 2048 per core).
Each core gets the full K/V for its batch (recomputed, cheap) and half the
query positions; output halves are concatenated on the host.

Device-side design (per core), all-fp8 (e4m3) matmul datapath:
  - Inputs, weights (BN scale folded on host), q/k/v activations and the
    exp(sim) tiles are all fp8e4m3; every big matmul runs in DoubleRow
    perf mode (256-deep contraction per instruction, 0.5 cycles/row).
    Measured end-to-end rel err of the full fp8 chain vs the fp32
    reference is ~5e-3 (threshold 2e-2).
  - sim is computed transposed (k on partitions, q on free dim) so the
    exp(sim) tiles feed the ctx matmul as the moving operand directly.
  - softmax uses a constant shift instead of a row max: exp(sim/sqrt(C) - 4)
    (sim/sqrt(C) is bounded by ~|q||k|/16 << 88, so no overflow), and the
    row sums come from a DoubleRow matmul with an all-ones fp8 stationary.
  - bias_v (which varies along the free dim of the vT tiles) is added via a
    rank-1 matmul (ones lhsT x bias rhs) accumulated into the same PSUM
    tile, so no separate DVE/ACT bias pass is needed.
  - ctx is accumulated unnormalized; normalization multiplies by 1/sums
    (reciprocal_approx_fast, ~18 bits) broadcast across partitions via a
    K=1 fp32 matmul with a ones column.
"""

import sys

for _p in ("/opt/trn_rl_repo", "/root/.axon_site/_ro/trn_rl_repo"):
    if _p not in sys.path:
        sys.path.insert(0, _p)

import ml_dtypes
import numpy as np

import concourse.bacc as bacc
import concourse.mybir as mybir
import concourse.tile as tile
from concourse.bass_utils import run_bass_kernel_spmd

F32 = mybir.dt.float32
F8 = mybir.dt.float8e4
AF = mybir.ActivationFunctionType
ALU = mybir.AluOpType
DR = mybir.MatmulPerfMode.DoubleRow
E4 = ml_dtypes.float8_e4m3

B, C, H, W = 4, 256, 64, 64
NK = H * W          # 4096 key positions per batch
NQ = NK // 2        # 2048 query positions per core
P = 128
CO = C // P         # 2 contraction subtiles (DoubleRow pair)
QC = 512            # query chunk (matmul moving free dim)
NQC = NQ // QC      # 4 query chunks per core
KT = NK // P        # 32 key tiles
NP = KT // 2        # 16 key-tile pairs
EXP_SHIFT = -4.0    # exp(sim/sqrt(C) + EXP_SHIFT); sim/sqrt(C) observed in [0.5, 7.7]
SCALE = 1.0 / np.sqrt(C)


def _build_program():
    nc = bacc.Bacc("TRN2", target_bir_lowering=False, debug=False)

    qf = nc.dram_tensor("qf", [C, NQ], F8, kind="ExternalInput").ap()
    kf = nc.dram_tensor("kf", [C, NK], F8, kind="ExternalInput").ap()
    wqT = nc.dram_tensor("wqT", [C, C], F8, kind="ExternalInput").ap()
    wkT = nc.dram_tensor("wkT", [C, C], F8, kind="ExternalInput").ap()
    wvT = nc.dram_tensor("wvT", [C, C], F8, kind="ExternalInput").ap()
    bq = nc.dram_tensor("bq", [P, CO], F32, kind="ExternalInput").ap()
    bk = nc.dram_tensor("bk", [P, CO], F32, kind="ExternalInput").ap()
    bv8 = nc.dram_tensor("bv8", [1, C], F8, kind="ExternalInput").ap()
    ones8 = nc.dram_tensor("ones8", [1, P], F8, kind="ExternalInput").ap()
    ones8p = nc.dram_tensor("ones8p", [P, 2 * 16], F8, kind="ExternalInput").ap()
    ones32 = nc.dram_tensor("ones32", [1, P], F32, kind="ExternalInput").ap()
    out = nc.dram_tensor("out", [C, NQ], F32, kind="ExternalOutput").ap()
    out_t = out.rearrange("(co ci) n -> ci co n", ci=P)

    with tile.TileContext(nc) as tc:
        with (
            nc.allow_low_precision(reason="fp8 matmul datapath (e4m3)"),
            tc.tile_pool(name="consts", bufs=1) as consts,
            tc.tile_pool(name="persist", bufs=1) as persist,
        ):
            # ---- constants (first projection only needs wqT + bq + qf) ----
            wqT_sb = consts.tile([P, CO, C], F8, name="wqT_sb")
            nc.gpsimd.dma_start(wqT_sb[:], wqT.rearrange("(co ci) o -> ci co o", ci=P))
            bq_sb = consts.tile([P, CO], F32, name="bq_sb")
            wkT_sb = consts.tile([P, CO, C], F8, name="wkT_sb")
            wvT_sb = consts.tile([P, CO, C], F8, name="wvT_sb")
            bk_sb = consts.tile([P, CO], F32, name="bk_sb")
            bv8_sb = consts.tile([1, C], F8, name="bv8_sb")
            ones8_sb = consts.tile([1, P], F8, name="ones8_sb")
            ones8p_sb = consts.tile([P, 2, 16], F8, name="ones8p_sb")
            ones32_sb = consts.tile([1, P], F32, name="ones32_sb")
            b0_sb = consts.tile([P, 1], F32, name="b0_sb")
            nc.vector.memset(b0_sb[:], EXP_SHIFT)
            # dummy activation: pulls the ~1.3us LoadActFuncSet into the
            # initial DMA-wait window instead of blocking the first relu
            warm_sb = consts.tile([P, 1], F32, name="warm_sb")
            nc.scalar.activation(warm_sb[:], b0_sb[:], AF.Relu)

            # ---- persistent activations (all fp8) ----
            q8_sb = persist.tile([P, CO, NQ], F8, name="q8_sb")
            k8_sb = persist.tile([P, CO, NK], F8, name="k8_sb")
            v8_sb = persist.tile([P, KT, C], F8, name="v8_sb")

            # ---- projections ----
            with (
                tc.tile_pool(name="staging", bufs=1) as staging,
                tc.tile_pool(name="proj_ps", bufs=1, space="PSUM") as proj_ps,
            ):
                qf_sb = staging.tile([P, CO, NQ], F8, name="qf_sb")
                qf_t = qf.rearrange("(co ci) n -> ci co n", ci=P)
                kf_sb = staging.tile([P, CO, NK], F8, name="kf_sb")
                kf_t = kf.rearrange("(co ci) n -> ci co n", ci=P)
                nc.gpsimd.dma_start(bq_sb[:], bq[:])
                nc.gpsimd.dma_start(wkT_sb[:], wkT.rearrange("(co ci) o -> ci co o", ci=P))
                nc.gpsimd.dma_start(bk_sb[:], bk[:])
                nc.gpsimd.dma_start(wvT_sb[:], wvT.rearrange("(co ci) o -> ci co o", ci=P))
                nc.gpsimd.dma_start(bv8_sb[:], bv8[:])
                nc.gpsimd.dma_start(ones8_sb[:], ones8[:])
                nc.gpsimd.dma_start(ones8p_sb[:], ones8p.rearrange("p (a b) -> p a b", a=2))
                nc.gpsimd.dma_start(ones32_sb[:], ones32[:])
                nc.sync.dma_start(qf_sb[:, :, :QC], qf_t[:, :, :QC])
                nc.sync.dma_start(qf_sb[:, :, QC:], qf_t[:, :, QC:])
                nc.sync.dma_start(kf_sb[:, :, :NK // 2], kf_t[:, :, :NK // 2])
                nc.sync.dma_start(kf_sb[:, :, NK // 2:], kf_t[:, :, NK // 2:])

                def proj_iter(j, w_sb, bias_sb, dst, src_sb):
                    # one [*, QC] chunk of a q/k projection; relu+bias for
                    # oo=0 runs on ACT, oo=1 on DVE so neither engine
                    # rate-limits PE
                    js = slice(j * QC, (j + 1) * QC)
                    for oo in range(CO):
                        ps = proj_ps.tile([P, QC], F32, tag="pj", bufs=2,
                                          name=f"ps_{j}_{oo}")
                        nc.tensor.matmul(
                            ps[:],
                            w_sb[:, :, oo * P:(oo + 1) * P],
                            src_sb[:, :, js],
                            start=True, stop=True, perf_mode=DR,
                        )
                        if oo == 0:
                            nc.scalar.activation(
                                dst[:, oo, js], ps[:], AF.Relu,
                                bias=bias_sb[:, oo:oo + 1],
                            )
                        else:
                            nc.vector.tensor_scalar(
                                dst[:, oo, js], ps[:],
                                bias_sb[:, oo:oo + 1], 0.0,
                                ALU.add, ALU.max,
                            )

                def vt_one(kt):
                    # vT[k, c] = relu(kf.T @ Wv'.T + bias_v): one DoubleRow
                    # matmul (256-deep contraction) + a rank-1 bias matmul
                    # accumulated into the same PSUM tile; relu alternates
                    # ACT/DVE.
                    psv = proj_ps.tile([P, C], F32, tag="pv", bufs=4,
                                       name=f"psv_{kt}")
                    nc.tensor.matmul(
                        psv[:],
                        kf_sb[:, :, kt * P:(kt + 1) * P],
                        wvT_sb[:, :, :],
                        start=True, stop=False, perf_mode=DR,
                    )
                    nc.tensor.matmul(
                        psv[:], ones8_sb[:], bv8_sb[:],
                        start=False, stop=True,
                    )
                    if kt % 2 == 0:
                        nc.scalar.activation(v8_sb[:, kt, :], psv[:], AF.Relu)
                    else:
                        nc.vector.tensor_scalar_max(v8_sb[:, kt, :], psv[:], 0.0)

                for j in range(NQ // QC):
                    proj_iter(j, wqT_sb, bq_sb, q8_sb, qf_sb)
                for j in range(NK // QC):
                    proj_iter(j, wkT_sb, bk_sb, k8_sb, kf_sb)
                    for kt in range(4 * j, 4 * j + 4):
                        vt_one(kt)

            # ---- attention ----
            with (
                tc.tile_pool(name="expp", bufs=1) as expp,
                tc.tile_pool(name="outp", bufs=1) as outp,
                tc.tile_pool(name="attn_ps", bufs=1, space="PSUM") as attn_ps,
            ):
                # Software pipeline: step s emits sim+exp for chunk s
                # interleaved (at k-pair granularity) with the ctx/sums
                # matmuls consuming chunk s-1's exp tiles. Adjacent exp pairs
                # are also merged (fp8 add) on the otherwise-idle Pool/DVE
                # engines so the row-sum matmuls only run per QUAD of key
                # tiles -- half the PE cost of summing per pair.
                NMQ = NP // 2          # 8 merged quads per chunk
                exp_pairs = {}         # qc -> list of pair tiles
                exp_quads = {}         # qc -> list of merged quad tiles

                def emit_sim_pair(qc, kp):
                    qs = slice(qc * QC, (qc + 1) * QC)
                    ps = attn_ps.tile([P, 2, QC], F32, tag="sim", bufs=2,
                                      name=f"pss_{qc}_{kp}")
                    for half in range(2):
                        kt = 2 * kp + half
                        nc.tensor.matmul(
                            ps[:, half, :],
                            k8_sb[:, :, kt * P:(kt + 1) * P],
                            q8_sb[:, :, qs],
                            start=True, stop=True, perf_mode=DR,
                        )
                    et = expp.tile([P, 2, QC], F8, tag="expT", bufs=20,
                                   name=f"expT_{qc}_{kp}")
                    nc.scalar.activation(et[:], ps[:], AF.Exp,
                                         bias=b0_sb[:], scale=float(SCALE))
                    pairs = exp_pairs.setdefault(qc, [])
                    pairs.append(et)
                    if kp % 2 == 1:
                        mg = expp.tile([P, 2, QC], F8, tag="mrg", bufs=10,
                                       name=f"mrg_{qc}_{kp // 2}")
                        nc.vector.tensor_tensor(mg[:], pairs[kp - 1][:],
                                                pairs[kp][:], ALU.add)
                        exp_quads.setdefault(qc, []).append(mg)

                def emit_ctx_pair(qc, kp, ctx_ps, sums_ps,
                                  cts=(0, 1), with_sums=True):
                    e = exp_pairs[qc][kp]
                    for ct in cts:
                        nc.tensor.matmul(
                            ctx_ps[ct][:],
                            v8_sb[:, 2 * kp:2 * kp + 2, ct * P:(ct + 1) * P],
                            e[:],
                            start=(kp == 0), stop=(kp == NP - 1),
                            perf_mode=DR, skip_group_check=True,
                        )
                    if with_sums and kp % 2 == 1:
                        mq = kp // 2
                        nc.tensor.matmul(
                            sums_ps[:], ones8p_sb[:, :, :2],
                            exp_quads[qc][mq][:],
                            start=(mq == 0), stop=(mq == NMQ - 1),
                            perf_mode=DR, skip_group_check=True,
                        )

                def emit_norm_chain(qc, sums_ps):
                    # recip -> broadcast -> copy; runs as soon as the sums
                    # accumulation stops, overlapping remaining ctx matmuls
                    recip = outp.tile([1, QC], F32, tag="recip", bufs=2,
                                      name=f"recip_{qc}")
                    nc.vector.reciprocal_approx_fast(recip[:], sums_ps[0:1, :])
                    bc_ps = attn_ps.tile([P, QC], F32, tag="bc", bufs=1,
                                         name=f"psb_{qc}")
                    nc.tensor.matmul(bc_ps[:], ones32_sb[:], recip[:],
                                     start=True, stop=True)
                    bc_sb = outp.tile([P, QC], F32, tag="bc", bufs=2,
                                      name=f"bc_{qc}")
                    nc.vector.tensor_copy(out=bc_sb[:], in_=bc_ps[:])
                    return bc_sb

                def emit_out(qc, ct, ctx_ps, bc_sb):
                    qs = slice(qc * QC, (qc + 1) * QC)
                    ot = outp.tile([P, QC], F32, tag="out", bufs=3,
                                   name=f"out_{qc}_{ct}")
                    nc.vector.tensor_mul(ot[:], ctx_ps[ct][:], bc_sb[:])
                    nc.sync.dma_start(out_t[:, ct, qs], ot[:])

                ctx_live = None  # (qc, ctx_ps, sums_ps) being accumulated
                for s in range(NQC + 1):
                    if s > 0:
                        qcp = s - 1
                        ctx_ps = [
                            attn_ps.tile([P, QC], F32, tag="ctx", bufs=2,
                                         name=f"psc_{qcp}_{ct}")
                            for ct in range(CO)
                        ]
                        sums_ps = attn_ps.tile([2, QC], F32, tag="sums", bufs=1,
                                               name=f"psS_{qcp}")
                        ctx_live = (qcp, ctx_ps, sums_ps)
                    if s < NQC:
                        # steady state: sim pairs interleaved with prev
                        # chunk's ctx pairs + quad sums
                        for kp in range(NP):
                            emit_sim_pair(s, kp)
                            if ctx_live is not None:
                                emit_ctx_pair(ctx_live[0], kp, ctx_live[1],
                                              ctx_live[2])
                        if ctx_live is not None:
                            qcp, ctx_ps, sums_ps = ctx_live
                            bc_sb = emit_norm_chain(qcp, sums_ps)
                            for ct in range(CO):
                                emit_out(qcp, ct, ctx_ps, bc_sb)
                            exp_pairs.pop(qcp)
                            exp_quads.pop(qcp)
                            ctx_live = None
                    else:
                        # drain step (no sim work left): run all quad sums
                        # first so the norm chain overlaps the ctx matmuls,
                        # then finish ct=0 completely so its output DMA
                        # overlaps ct=1's matmuls.
                        qcp, ctx_ps, sums_ps = ctx_live
                        for mq in range(NMQ):
                            nc.tensor.matmul(
                                sums_ps[:], ones8p_sb[:, :, :2],
                                exp_quads[qcp][mq][:],
                                start=(mq == 0), stop=(mq == NMQ - 1),
                                perf_mode=DR, skip_group_check=True,
                            )
                        bc_sb = emit_norm_chain(qcp, sums_ps)
                        for ct in range(CO):
                            for kp in range(NP):
                                emit_ctx_pair(qcp, kp, ctx_ps, sums_ps,
                                              cts=(ct,), with_sums=False)
                            emit_out(qcp, ct, ctx_ps, bc_sb)
                        exp_pairs.pop(qcp)
                        exp_quads.pop(qcp)
                        ctx_live = None

    nc.compile()
    return nc


_PROGRAM = None


def _get_program():
    global _PROGRAM
    if _PROGRAM is None:
        _PROGRAM = _build_program()
    return _PROGRAM


def _prepare_in_maps(
    query_feats, key_feats, Wq, Wk, Wv,
    scale_q, bias_q, scale_k, bias_k, scale_v, bias_v,
):
    f32 = np.float32
    qf_all = np.asarray(query_feats, f32).reshape(B, C, NK)
    kf_all = np.asarray(key_feats, f32).reshape(B, C, NK)

    wqT = np.ascontiguousarray(
        (np.asarray(scale_q, f32)[:, None] * np.asarray(Wq, f32)).T).astype(E4)
    wkT = np.ascontiguousarray(
        (np.asarray(scale_k, f32)[:, None] * np.asarray(Wk, f32)).T).astype(E4)
    wvT = np.ascontiguousarray(
        (np.asarray(scale_v, f32)[:, None] * np.asarray(Wv, f32)).T).astype(E4)
    bq2 = np.ascontiguousarray(np.asarray(bias_q, f32).reshape(CO, P).T)
    bk2 = np.ascontiguousarray(np.asarray(bias_k, f32).reshape(CO, P).T)
    bv8 = np.asarray(bias_v, f32)[None, :].astype(E4)
    ones8 = np.ones((1, P), E4)
    ones8p = np.ones((P, 2 * 16), E4)
    ones32 = np.ones((1, P), f32)

    shared = dict(wqT=wqT, wkT=wkT, wvT=wvT, bq=bq2, bk=bk2,
                  bv8=bv8, ones8=ones8, ones8p=ones8p, ones32=ones32)
    in_maps = []
    for core in range(8):
        b, h = divmod(core, 2)
        in_maps.append(dict(
            qf=np.ascontiguousarray(
                qf_all[b][:, h * NQ:(h + 1) * NQ]).astype(E4),
            kf=np.ascontiguousarray(kf_all[b]).astype(E4),
            **shared,
        ))
    return in_maps


def run(inputs: dict, trace: bool = False):
    """Compile (cached) + run on 8 cores. Returns (output, BassKernelResults)."""
    nc = _get_program()
    in_maps = _prepare_in_maps(**inputs)
    res = run_bass_kernel_spmd(nc, in_maps, core_ids=list(range(8)), trace=trace)
    full = np.empty((B, C, NK), np.float32)
    for core in range(8):
        b, h = divmod(core, 2)
        full[b][:, h * NQ:(h + 1) * NQ] = res.results[core]["out"]
    return full.reshape(B, C, H, W), res


def kernel(**inputs) -> np.ndarray:
    return run(inputs)[0]
